# revision 9
# baseline (speedup 1.0000x reference)
"""Dilated-attention transformer block kernel for TRN2, 8-core SPMD. v3.

Sharding: (batch b in {0,1}) x (sequence chunk c in {0..3}) -> 8 cores.
Each core computes the full block for its 512 tokens, with a 512-token
halo for K/V. Dilation=2 handled by parity-grouping the sequence.

v3 precision plan (vs v2's all-fp8-hi/lo):
  - QKV GEMM: fp8 e4m3 single-plane weights, DoubleRow (1x cost).
  - Attention internals (q/k/v/probs) in bf16: scores contraction is 64
    so bf16 costs the same MMs as DR-fp8, with none of the fp8 noise.
  - Mask applied as a bf16 0/1 multiply on DVE after exp (no mask MMs).
  - out-proj + FFN1 in bf16 (hi/lo fp8 costs the same as bf16, is worse).
  - FFN2: fp8 single-plane DR (FFN2_MODE='bf16' fallback).
"""
import os
import numpy as np
import ml_dtypes
import concourse.bass as bass
from concourse import bacc
import concourse.mybir as mybir
from concourse.tile import TileContext
from concourse.bass_utils import run_bass_kernel_spmd
from concourse.masks import make_identity

dt = mybir.dt
F32, F32R, F8, BF16 = dt.float32, dt.float32r, dt.float8e4, dt.bfloat16
AF = mybir.ActivationFunctionType
OP = mybir.AluOpType
PM = mybir.MatmulPerfMode
E4 = ml_dtypes.float8_e4m3

B, L, D, H, HD = 2, 2048, 1024, 16, 64
SCALE = 1.0 / float(np.sqrt(HD))
WS = 64.0          # fp8 weight quantization scale (power of 2)
VS = 32.0          # Wv quantization scale; V tiles hold 32*v, sum row = 32

NVQ = 4            # qkv DR pairs (single-plane)
NV2 = 16           # ffn2 DR pairs (single-plane)
TDT = F32          # transpose dtype
FFN2_MODE = os.environ.get("FFN2_MODE", "f8")  # 'f8' | 'bf16'
KPHASE = int(os.environ.get("KPHASE", "5"))    # build-bisect: 1..5


def build(reps: int = 1):
    nc = bacc.Bacc(None, target_bir_lowering=False)
    xg_d = nc.declare_dram_parameter("xg", [1024, 1024], BF16, isOutput=False)
    xt_d = nc.declare_dram_parameter("xt", [1024, 1024], BF16, isOutput=False)
    wq_d = nc.declare_dram_parameter("wq", [NVQ * 128, 2, 1024], F8, isOutput=False)
    wk_d = nc.declare_dram_parameter("wk", [NVQ * 128, 2, 1024], F8, isOutput=False)
    wv_d = nc.declare_dram_parameter("wv", [NVQ * 128, 2, 1024], F8, isOutput=False)
    wo_d = nc.declare_dram_parameter("wo", [1024, 1024], BF16, isOutput=False)
    w1_d = nc.declare_dram_parameter("w1", [1024, 4096], BF16, isOutput=False)
    if FFN2_MODE == "f8":
        w2_d = nc.declare_dram_parameter("w2", [NV2 * 128, 2, 1024], F8,
                                         isOutput=False)
    else:
        w2_d = nc.declare_dram_parameter("w2", [4096, 1024], BF16,
                                         isOutput=False)
    bq_d = nc.declare_dram_parameter("bq", [128, 8], F32, isOutput=False)
    bk_d = nc.declare_dram_parameter("bk", [128, 8], F32, isOutput=False)
    b1_d = nc.declare_dram_parameter("b1", [128, 32], F32, isOutput=False)
    bv32_d = nc.declare_dram_parameter("bv32", [1, 1024], F32, isOutput=False)
    bout_d = nc.declare_dram_parameter("bout", [1, 1024], F32, isOutput=False)
    b2_d = nc.declare_dram_parameter("b2", [1, 1024], F32, isOutput=False)
    kmask_d = nc.declare_dram_parameter("kmask", [128, 4, 256], BF16,
                                        isOutput=False)
    out_d = nc.declare_dram_parameter("out", [512, 1024], F32, isOutput=True)

    with TileContext(nc) as tc:
        with tc.tile_pool(name="const", bufs=1) as pconst, \
             tc.tile_pool(name="glob", bufs=1) as glob, \
             tc.tile_pool(name="rot", bufs=2) as rot, \
             tc.tile_pool(name="small", bufs=4) as small:
            # ---- constants ----
            identb = pconst.tile([128, 128], TDT, tag="identb", name="identb")
            make_identity(nc, identb[:])
            kmask = pconst.tile([128, 4, 256], BF16, tag="kmask", name="kmask")
            nc.sync.dma_start(out=kmask[:], in_=kmask_d[:])
            bq_t = pconst.tile([128, 8], F32, tag="bq", name="bq")
            bk_t = pconst.tile([128, 8], F32, tag="bk", name="bk")
            b1_t = pconst.tile([128, 32], F32, tag="b1", name="b1")
            nc.sync.dma_start(out=bq_t[:], in_=bq_d[:])
            nc.sync.dma_start(out=bk_t[:], in_=bk_d[:])
            nc.sync.dma_start(out=b1_t[:], in_=b1_d[:])
            bv32_bc = pconst.tile([128, 1024], F32, tag="bv32bc", name="bv32bc")
            bout_bc = pconst.tile([128, 1024], F32, tag="boutbc", name="boutbc")
            b2_bc = pconst.tile([128, 1024], F32, tag="b2bc", name="b2bc")
            brow = pconst.tile([1, 1024], F32, tag="brow", name="brow")
            for src, dst in ((bv32_d, bv32_bc), (bout_d, bout_bc), (b2_d, b2_bc)):
                nc.sync.dma_start(out=brow[:], in_=src[:])
                nc.gpsimd.partition_broadcast(dst[:], brow[:])
            eps_t = pconst.tile([128, 1], F32, tag="eps", name="eps")
            nc.vector.memset(eps_t[:], 1e-5)

            xnew = [glob.tile([128, 1024], F32, tag=f"xn{t}", name=f"xn{t}")
                    for t in range(4)]
            if KPHASE < 4:
                for t in range(4):
                    nc.vector.memset(xnew[t][:], 0.0)


            for _rep in range(reps):
                with tc.tile_pool(name="attn", bufs=1) as attn:
                    # hTp[par][j]: [128, 2, 512] fp8 DR pairs of LN1 output
                    hTp = [[attn.tile([128, 2, 512], F8, tag=f"hT{par}_{j}",
                                      name=f"hT{par}_{j}") for j in range(4)]
                           for par in range(2)]
                    # qT[m]: [128=2heads x 64hd, 2par x 256q] bf16
                    qT8 = [attn.tile([128, 512], BF16, tag=f"qT{m}",
                                     name=f"qT{m}") for m in range(8)]
                    # kT[m]: [128, 2par x 512keys] bf16
                    kT8 = [attn.tile([128, 1024], BF16, tag=f"kT{m}",
                                     name=f"kT{m}") for m in range(8)]
                    # Vb[par][kt]: [128 keys, 16 heads, 66] bf16 (col 64:66=VS)
                    Vb = [[attn.tile([128, 16, 66], BF16, tag=f"V{par}_{kt}",
                                     name=f"V{par}_{kt}") for kt in range(4)]
                          for par in range(2)]
                    for par in range(2):
                        for kt in range(4):
                            nc.vector.memset(Vb[par][kt][:, :, 64:66], VS)
                    # oT[d]: [128 dout, 2par x 256 tok] bf16
                    oT8 = [attn.tile([128, 512], BF16, tag=f"oT{d}",
                                     name=f"oT{d}") for d in range(8)]

                    # ---- LN1: stats from xg, normalize host-packed xT ----
                    # stt cols 0..7 = mu*rstd per token tile, 8..15 = rstd
                    stt = attn.tile([128, 16], F32, tag="stt", name="stt")
                    xgr = xg_d.rearrange("(t p) d -> t p d", p=128)
                    xtr = xt_d.rearrange("(s p) d -> s p d", p=128)
                    xg_own = {}
                    with tc.tile_pool(name="halo", bufs=1) as halo, \
                         tc.tile_pool(name="ppst", bufs=1, space="PSUM") as ppst:
                        # prefetch xT slices on the ACT DMA queue (keeps the
                        # sync queue free for weight streams)
                        xts8 = [halo.tile([128, 1024], BF16, tag=f"xts{s}",
                                          name=f"xts{s}") for s in range(8)]
                        for s in range(8):
                            nc.scalar.dma_start(out=xts8[s][:], in_=xtr[s])
                        for t in range(8):
                            par, tt = t // 4, t % 4
                            if tt >= 2:
                                xt = attn.tile([128, 1024], BF16,
                                               tag=f"xg{t}", name=f"xg{t}")
                                xg_own[t] = xt
                            else:
                                xt = halo.tile([128, 1024], BF16,
                                               tag=f"xh{t}", name=f"xh{t}")
                            nc.sync.dma_start(out=xt[:], in_=xgr[t])
                            stats = small.tile(
                                [128, 2, nc.vector.BN_STATS_DIM], F32,
                                tag="stats", name="stats")
                            mv = small.tile([128, nc.vector.BN_AGGR_DIM],
                                            F32, tag="mv", name="mv")
                            for sg in range(2):
                                nc.vector.bn_stats(
                                    out=stats[:, sg, :],
                                    in_=xt[:, sg * 512:(sg + 1) * 512])
                            nc.vector.bn_aggr(out=mv[:], in_=stats[:])
                            nc.scalar.activation(out=stt[:, 8 + t:9 + t],
                                                 in_=mv[:, 1:2],
                                                 func=AF.Sqrt, bias=eps_t[:],
                                                 scale=1.0)
                            nc.vector.reciprocal(out=stt[:, 8 + t:9 + t],
                                                 in_=stt[:, 8 + t:9 + t])
                            nc.vector.tensor_tensor(out=stt[:, t:t + 1],
                                                    in0=mv[:, 0:1],
                                                    in1=stt[:, 8 + t:9 + t],
                                                    op=OP.mult)
                        pst = ppst.tile([16, 128], F32, tag="pst", name="pst")
                        nc.tensor.matmul(pst[:], stt[:], identb[:],
                                         is_transpose=True, start=True,
                                         stop=True)
                        sttT = attn.tile([16, 128], F32, tag="sttT",
                                         name="sttT")
                        nc.scalar.activation(out=sttT[:], in_=pst[:],
                                             func=AF.Copy)
                        statrow = attn.tile([1, 16, 128], F32, tag="statrow",
                                            name="statrow")
                        nc.scalar.dma_start(out=statrow[:], in_=sttT[:])
                        mur_bc = attn.tile([128, 1024], F32, tag="murbc",
                                           name="murbc")
                        r_bc = attn.tile([128, 1024], F32, tag="rbc1",
                                         name="rbc1")
                        nc.gpsimd.partition_broadcast(mur_bc[:],
                                                      statrow[0:1, 0:8, :])
                        nc.gpsimd.partition_broadcast(r_bc[:],
                                                      statrow[0:1, 8:16, :])
                        for s in range(8):
                            t1 = rot.tile([128, 1024], F32, tag="t1",
                                          name="t1")
                            nc.vector.tensor_tensor(out=t1[:],
                                                    in0=xts8[s][:],
                                                    in1=r_bc[:], op=OP.mult)
                            for par in range(2):
                                nc.vector.tensor_tensor(
                                    out=hTp[par][s // 2][:, s % 2, :],
                                    in0=t1[:, par * 512:(par + 1) * 512],
                                    in1=mur_bc[:, par * 512:(par + 1) * 512],
                                    op=OP.subtract)

                    # ---- QKV (fp8 single-plane DR; bf16 outputs) ----
                    if KPHASE >= 2:
                      with tc.tile_pool(name="pqkv", bufs=2, space="PSUM") as pq, \
                           tc.tile_pool(name="wm", bufs=1) as wm:
                        wqt = wm.tile([128, NVQ, 2, 1024], F8, tag="wq",
                                      name="wq")
                        wkt = wm.tile([128, NVQ, 2, 1024], F8, tag="wk",
                                      name="wk")
                        wvt = wm.tile([128, NVQ, 2, 1024], F8, tag="wv",
                                      name="wv")
                        for w_d, w_t in ((wq_d, wqt), (wk_d, wkt), (wv_d, wvt)):
                            nc.sync.dma_start(
                                out=w_t[:],
                                in_=w_d.rearrange("(v p) i c -> p v i c",
                                                  p=128))
                        for m in range(8):
                            for par in range(2):
                                psq = pq.tile([128, 256], F32, tag="psq",
                                              name="psq")
                                for v in range(NVQ):
                                    nc.tensor.matmul(
                                        psq[:],
                                        wqt[:, v, :, m * 128:(m + 1) * 128],
                                        hTp[par][v][:, :, 256:512],
                                        start=(v == 0), stop=(v == NVQ - 1),
                                        perf_mode=PM.DoubleRow)
                                nc.vector.tensor_scalar(
                                    out=qT8[m][:, par * 256:(par + 1) * 256],
                                    in0=psq[:], scalar1=1.0 / WS,
                                    scalar2=bq_t[:, m:m + 1],
                                    op0=OP.mult, op1=OP.add)
                        for m in range(8):
                            for par in range(2):
                                psk = pq.tile([128, 512], F32, tag="psk",
                                              name="psk")
                                for v in range(NVQ):
                                    nc.tensor.matmul(
                                        psk[:],
                                        wkt[:, v, :, m * 128:(m + 1) * 128],
                                        hTp[par][v][:],
                                        start=(v == 0), stop=(v == NVQ - 1),
                                        perf_mode=PM.DoubleRow)
                                nc.vector.tensor_scalar(
                                    out=kT8[m][:, par * 512:(par + 1) * 512],
                                    in0=psk[:], scalar1=1.0 / WS,
                                    scalar2=bk_t[:, m:m + 1],
                                    op0=OP.mult, op1=OP.add)
                        for npass in range(2):
                            for par in range(2):
                                for tt in range(4):
                                    psv = pq.tile([128, 512], F32, tag="psv",
                                                  name="psv")
                                    for v in range(NVQ):
                                        nc.tensor.matmul(
                                            psv[:],
                                            hTp[par][v][:, :,
                                                        tt * 128:(tt + 1) * 128],
                                            wvt[:, v, :,
                                                npass * 512:(npass + 1) * 512],
                                            start=(v == 0), stop=(v == NVQ - 1),
                                            perf_mode=PM.DoubleRow)
                                    nc.vector.tensor_tensor(
                                        out=Vb[par][tt][:, npass * 8:
                                                        (npass + 1) * 8, 0:64],
                                        in0=psv[:].rearrange("p (h e) -> p h e",
                                                             h=8),
                                        in1=bv32_bc[:, npass * 512:(npass + 1) * 512]
                                            .rearrange("p (h e) -> p h e", h=8),
                                        op=OP.add)

                    # ---- attention (bf16 internals, mask on DVE) ----
                    for par in range(2 if KPHASE >= 3 else 0):
                        with tc.tile_pool(name=f"ppS{par}", bufs=2,
                                          space="PSUM") as ppS, \
                             tc.tile_pool(name=f"ppO{par}", bufs=4,
                                          space="PSUM") as ppO, \
                             tc.tile_pool(name=f"ex{par}", bufs=3) as ex:
                            for h in range(16):
                                m, a = h // 2, h % 2
                                pso = ppO.tile([66, 256], F32, tag="pso",
                                               name="pso")
                                pss = ppS.tile([128, 4, 256], F32, tag="pss",
                                               name="pss")
                                for kt in range(4):
                                    nc.tensor.matmul(
                                        pss[:, kt, :],
                                        kT8[m][a * 64:(a + 1) * 64,
                                               par * 512 + kt * 128:
                                               par * 512 + (kt + 1) * 128],
                                        qT8[m][a * 64:(a + 1) * 64,
                                               par * 256:(par + 1) * 256],
                                        start=True, stop=True)
                                expm = ex.tile([128, 4, 256], BF16, tag="expm",
                                               name="expm")
                                nc.scalar.activation(out=expm[:], in_=pss[:],
                                                     func=AF.Exp, scale=SCALE)
                                nc.vector.tensor_tensor(
                                    out=expm[:], in0=expm[:], in1=kmask[:],
                                    op=OP.mult)
                                for kt in range(4):
                                    nc.tensor.matmul(
                                        pso[:],
                                        Vb[par][kt][:, h, :],
                                        expm[:, kt, :],
                                        start=(kt == 0), stop=(kt == 3))
                                rec = small.tile([1, 256], F32, tag="rec",
                                                 name="rec")
                                nc.vector.reciprocal(out=rec[:],
                                                     in_=pso[64:65, :])
                                rbc = small.tile([64, 256], F32,
                                                 tag="rbc", name="rbc")
                                nc.gpsimd.partition_broadcast(rbc[:], rec[:])
                                nc.vector.tensor_tensor(
                                    out=oT8[h // 2][64 * (h % 2):
                                                    64 * (h % 2) + 64,
                                                    par * 256:(par + 1) * 256],
                                    in0=pso[0:64, :],
                                    in1=rbc[:], op=OP.mult)

                    # ---- out-proj (bf16) + residual ----
                    if KPHASE >= 4:
                      with tc.tile_pool(name="pp8", bufs=1, space="PSUM") as pp8, \
                           tc.tile_pool(name="wr2", bufs=3) as wr2:
                        pso_ = [pp8.tile([128, 512], F32, tag=f"po{i}",
                                         name=f"po{i}") for i in range(8)]
                        for v in range(8):
                            wo = wr2.tile([128, 1024], BF16, tag="wo", name="wo")
                            nc.sync.dma_start(
                                out=wo[:], in_=wo_d[v * 128:(v + 1) * 128])
                            for tb in range(4):
                                for npass in range(2):
                                    nc.tensor.matmul(
                                        pso_[tb * 2 + npass][:],
                                        oT8[v][:, tb * 128:(tb + 1) * 128],
                                        wo[:, npass * 512:(npass + 1) * 512],
                                        start=(v == 0), stop=(v == 7))
                        for tb in range(4):
                            xob = small.tile([128, 1024], F32, tag="xob",
                                             bufs=2, name="xob")
                            xg_o = xg_own[(tb // 2) * 4 + 2 + tb % 2]
                            nc.gpsimd.tensor_tensor(out=xob[:], in0=xg_o[:],
                                                    in1=bout_bc[:], op=OP.add)
                            for npass in range(2):
                                nc.vector.tensor_tensor(
                                    out=xnew[tb][:, npass * 512:(npass + 1) * 512],
                                    in0=pso_[tb * 2 + npass][:],
                                    in1=xob[:, npass * 512:(npass + 1) * 512],
                                    op=OP.add)

                # ---- FFN ----
                if KPHASE >= 5:
                  with tc.tile_pool(name="ffn", bufs=1) as ffn:
                    # h2T[s]: [128, 512] bf16 (LN2 output, transposed)
                    h2T = [ffn.tile([128, 512], BF16, tag=f"h2T{s}",
                                    name=f"h2T{s}") for s in range(8)]
                    if FFN2_MODE == "f8":
                        fTp = [ffn.tile([128, 2, 512], F8, tag=f"fT{j}",
                                        name=f"fT{j}") for j in range(16)]
                    else:
                        fTb = [ffn.tile([128, 512], BF16, tag=f"fTb{m}",
                                        name=f"fTb{m}") for m in range(32)]
                    with tc.tile_pool(name="ppT2", bufs=8, space="PSUM") as ppT2:
                        pt4s = [ppT2.tile([128, 4, 128], TDT, tag="pt",
                                          name=f"pt2_{d}") for d in range(8)]
                        for t in range(4):
                            h2 = rot.tile([128, 1024], TDT, tag="h", name="h2")
                            stats = small.tile([128, 2, nc.vector.BN_STATS_DIM],
                                               F32, tag="stats", name="stats")
                            mv = small.tile([128, nc.vector.BN_AGGR_DIM], F32,
                                            tag="mv", name="mv")
                            rstd = small.tile([128, 1], F32, tag="rstd",
                                              name="rstd")
                            for sg in range(2):
                                nc.vector.bn_stats(
                                    out=stats[:, sg, :],
                                    in_=xnew[t][:, sg * 512:(sg + 1) * 512])
                            nc.vector.bn_aggr(out=mv[:], in_=stats[:])
                            nc.scalar.activation(out=rstd[:], in_=mv[:, 1:2],
                                                 func=AF.Sqrt, bias=eps_t[:],
                                                 scale=1.0)
                            nc.vector.reciprocal(out=rstd[:], in_=rstd[:])
                            nc.vector.tensor_scalar(
                                out=h2[:], in0=xnew[t][:], scalar1=mv[:, 0:1],
                                scalar2=rstd[:], op0=OP.subtract, op1=OP.mult)
                            # xnew += b2 (residual base for FFN2, post-stats)
                            nc.gpsimd.tensor_tensor(out=xnew[t][:],
                                                    in0=xnew[t][:],
                                                    in1=b2_bc[:], op=OP.add)
                            for d in range(8):
                                nc.tensor.matmul(
                                    pt4s[d][:, t, :],
                                    h2[:, d * 128:(d + 1) * 128],
                                    identb[:], is_transpose=True,
                                    start=(t == 0), stop=(t == 3))
                        for d in range(8):
                            nc.scalar.activation(
                                out=h2T[d][:],
                                in_=pt4s[d][:].rearrange("p a b -> p (a b)"),
                                func=AF.Copy)

                    with tc.tile_pool(name="pf1", bufs=4, space="PSUM") as pf1, \
                         tc.tile_pool(name="wm2", bufs=2) as wm2:
                        w1r = w1_d.rearrange("(s p) m -> p s m", p=128)
                        for mc in range(4):
                            w1t = wm2.tile([128, 8, 1024], BF16, tag="w1",
                                           name="w1")
                            nc.sync.dma_start(
                                out=w1t[:],
                                in_=w1r[:, :, mc * 1024:(mc + 1) * 1024])
                            for mi in range(8):
                                m = mc * 8 + mi
                                ps = pf1.tile([128, 512], F32, tag="ps",
                                              name="ps")
                                for s in range(8):
                                    nc.tensor.matmul(
                                        ps[:],
                                        w1t[:, s, mi * 128:(mi + 1) * 128],
                                        h2T[s][:],
                                        start=(s == 0), stop=(s == 7))
                                if FFN2_MODE == "f8":
                                    nc.scalar.activation(
                                        out=fTp[m // 2][:, m % 2, :], in_=ps[:],
                                        func=AF.Gelu, bias=b1_t[:, m:m + 1],
                                        scale=1.0)
                                else:
                                    nc.scalar.activation(
                                        out=fTb[m][:], in_=ps[:],
                                        func=AF.Gelu, bias=b1_t[:, m:m + 1],
                                        scale=1.0)

                    with tc.tile_pool(name="pp8b", bufs=1, space="PSUM") as pp8b, \
                         tc.tile_pool(name="wr3", bufs=3) as wr3:
                        psf = [pp8b.tile([128, 512], F32, tag=f"pf{i}",
                                         name=f"pf{i}") for i in range(8)]
                        if FFN2_MODE == "f8":
                            for j in range(NV2):
                                w2t = wr3.tile([128, 2, 1024], F8, tag="w2",
                                               name="w2")
                                nc.sync.dma_start(
                                    out=w2t[:], in_=w2_d[j * 128:(j + 1) * 128])
                                for tb in range(4):
                                    for npass in range(2):
                                        nc.tensor.matmul(
                                            psf[tb * 2 + npass][:],
                                            fTp[j][:, :, tb * 128:(tb + 1) * 128],
                                            w2t[:, :, npass * 512:(npass + 1) * 512],
                                            start=(j == 0), stop=(j == NV2 - 1),
                                            perf_mode=PM.DoubleRow)
                        else:
                            for j in range(32):
                                w2t = wr3.tile([128, 1024], BF16, tag="w2",
                                               name="w2")
                                nc.sync.dma_start(
                                    out=w2t[:], in_=w2_d[j * 128:(j + 1) * 128])
                                for tb in range(4):
                                    for npass in range(2):
                                        nc.tensor.matmul(
                                            psf[tb * 2 + npass][:],
                                            fTb[j][:, tb * 128:(tb + 1) * 128],
                                            w2t[:, npass * 512:(npass + 1) * 512],
                                            start=(j == 0), stop=(j == 31))
                        f2scale = 1.0 / WS if FFN2_MODE == "f8" else 1.0
                        for tb in range(4):
                            for npass in range(2):
                                if FFN2_MODE == "f8":
                                    tmp = small.tile([128, 512], F32, tag="tmp",
                                                     bufs=2, name="tmpf")
                                    nc.scalar.activation(
                                        out=tmp[:], in_=psf[tb * 2 + npass][:],
                                        func=AF.Copy, scale=f2scale)
                                    src = tmp[:]
                                else:
                                    src = psf[tb * 2 + npass][:]
                                nc.vector.tensor_tensor(
                                    out=xnew[tb][:, npass * 512:(npass + 1) * 512],
                                    in0=src,
                                    in1=xnew[tb][:, npass * 512:(npass + 1) * 512],
                                    op=OP.add)

            for t in range(4):
                nc.sync.dma_start(out=out_d.rearrange("(t p) d -> t p d", p=128)[t],
                                  in_=xnew[t][:])

    nc.compile()
    return nc


# ---------------- host-side packing ----------------

def _q8(a):
    return np.clip(a, -224.0, 224.0).astype(E4)


def _pack_hi(w, nv):
    """w [K, M] fp32 (pre-scaled) -> [nv*128, 2, M] fp8 DR pair planes
    (single plane, no compensation). Pair j covers rows 2j*128..(2j+2)*128."""
    K, M = w.shape
    assert K == nv * 256
    hi = _q8(w)
    out = np.zeros((nv * 128, 2, M), E4)
    for j in range(nv):
        for i in range(2):
            out[j * 128:(j + 1) * 128, i, :] = hi[(2 * j + i) * 128:
                                                  (2 * j + i + 1) * 128, :]
    return out


def _make_kmask(c):
    """bf16 keep-indicator [128 kk, 4 kt, 256 q] for chunk c (parity space)."""
    kk = np.arange(128)[:, None]
    q = np.arange(256)[None, :]
    km = np.zeros((128, 4, 256), np.float32)
    for t in range(4):
        Qg = c * 256 + q
        Kg = c * 256 - 256 + t * 128 + kk
        keep = (Kg >= 0) & (Qg - Kg >= 0) & (Qg - Kg <= 256)
        km[:, t, :] = keep.astype(np.float32)
    return km.astype(ml_dtypes.bfloat16)


def make_in_maps(inputs):
    x = np.asarray(inputs["x"], np.float32)
    ln1g = np.asarray(inputs["ln1_g"], np.float32)
    ln1b = np.asarray(inputs["ln1_b"], np.float32)
    ln2g = np.asarray(inputs["ln2_g"], np.float32)
    ln2b = np.asarray(inputs["ln2_b"], np.float32)
    Wqkv = np.asarray(inputs["Wqkv"], np.float32)
    bqkv = np.asarray(inputs["bqkv"], np.float32)
    Wout = np.asarray(inputs["Wout"], np.float32)
    bout = np.asarray(inputs["bout"], np.float32)
    W1 = np.asarray(inputs["W1"], np.float32)
    b1 = np.asarray(inputs["b1"], np.float32)
    W2 = np.asarray(inputs["W2"], np.float32)
    b2 = np.asarray(inputs["b2"], np.float32)

    # fold LN1 gain/bias into Wqkv/bqkv, LN2 into W1/b1
    Wqkv_f = Wqkv * ln1g[:, None]
    bqkv_f = bqkv + ln1b @ Wqkv
    W1_f = W1 * ln2g[:, None]
    b1_f = b1 + ln2b @ W1

    wq_planes = _pack_hi(Wqkv_f[:, :1024] * WS, NVQ)
    wk_planes = _pack_hi(Wqkv_f[:, 1024:2048] * WS, NVQ)
    wv_planes = _pack_hi(Wqkv_f[:, 2048:] * VS, NVQ)
    bq = bqkv_f[:1024].reshape(8, 128).T.copy()
    bk = bqkv_f[1024:2048].reshape(8, 128).T.copy()
    bv32 = (bqkv_f[2048:] * VS).reshape(1, 1024)
    if FFN2_MODE == "f8":
        w2_planes = _pack_hi(W2 * WS, NV2)
    else:
        w2_planes = W2.astype(ml_dtypes.bfloat16)

    common = {
        "wq": wq_planes, "wk": wk_planes, "wv": wv_planes,
        "wo": Wout.astype(ml_dtypes.bfloat16),
        "w1": W1_f.astype(ml_dtypes.bfloat16),
        "w2": w2_planes,
        "bq": np.ascontiguousarray(bq), "bk": np.ascontiguousarray(bk),
        "b1": np.ascontiguousarray(b1_f.reshape(32, 128).T),
        "bv32": bv32, "bout": bout.reshape(1, 1024), "b2": b2.reshape(1, 1024),
    }
    in_maps = []
    for core in range(8):
        b, c = core // 4, core % 4
        xg = np.zeros((1024, 1024), np.float32)
        for par in range(2):
            i0, i1 = c * 256 - 256, c * 256 + 256
            ii = np.arange(max(i0, 0), i1)
            xg[par * 512 + (ii - i0), :] = x[b, 2 * ii + par, :]
        m = dict(common)
        m["xg"] = xg.astype(ml_dtypes.bfloat16)
        m["xt"] = np.ascontiguousarray(xg.T).astype(ml_dtypes.bfloat16)
        m["kmask"] = _make_kmask(c)
        in_maps.append(m)
    return in_maps


def assemble(results):
    out = np.zeros((B, L, D), np.float32)
    for core in range(8):
        b, c = core // 4, core % 4
        o = results[core]["out"]
        for par in range(2):
            ii = np.arange(c * 256, (c + 1) * 256)
            out[b, 2 * ii + par, :] = o[par * 256:(par + 1) * 256, :]
    return out


_CACHE = {}


def kernel(**inputs):
    """Full-input entry point: shards across 8 NeuronCores, runs the Bass
    kernel SPMD, gathers the full [B, L, D] float32 output."""
    if "nc" not in _CACHE:
        _CACHE["nc"] = build()
    nc = _CACHE["nc"]
    in_maps = make_in_maps(inputs)
    res = run_bass_kernel_spmd(nc, in_maps, list(range(8)))
    return assemble(res.results)


# revision 10
# speedup vs baseline: 1.3646x; 1.3646x over previous
"""Dilated-attention transformer block kernel for TRN2, 8-core SPMD. v3.

Sharding: (batch b in {0,1}) x (sequence chunk c in {0..3}) -> 8 cores.
Each core computes the full block for its 512 tokens, with a 512-token
halo for K/V. Dilation=2 handled by parity-grouping the sequence.

v3 precision plan (vs v2's all-fp8-hi/lo):
  - QKV GEMM: fp8 e4m3 single-plane weights, DoubleRow (1x cost).
  - Attention internals (q/k/v/probs) in bf16: scores contraction is 64
    so bf16 costs the same MMs as DR-fp8, with none of the fp8 noise.
  - Mask applied as a bf16 0/1 multiply on DVE after exp (no mask MMs).
  - out-proj + FFN1 in bf16 (hi/lo fp8 costs the same as bf16, is worse).
  - FFN2: fp8 single-plane DR (FFN2_MODE='bf16' fallback).
"""
import os
import numpy as np
import ml_dtypes
import concourse.bass as bass
from concourse import bacc
import concourse.mybir as mybir
from concourse.tile import TileContext
from concourse.bass_utils import run_bass_kernel_spmd
from concourse.masks import make_identity

dt = mybir.dt
F32, F32R, F8, BF16 = dt.float32, dt.float32r, dt.float8e4, dt.bfloat16
AF = mybir.ActivationFunctionType
OP = mybir.AluOpType
PM = mybir.MatmulPerfMode
E4 = ml_dtypes.float8_e4m3

B, L, D, H, HD = 2, 2048, 1024, 16, 64
SCALE = 1.0 / float(np.sqrt(HD))
WS = 64.0          # fp8 weight quantization scale (power of 2)
VS = 32.0          # Wv quantization scale; V tiles hold 32*v, sum row = 32

NVQ = 4            # qkv DR pairs (single-plane)
NV2 = 16           # ffn2 DR pairs (single-plane)
TDT = F32          # transpose dtype
FFN2_MODE = os.environ.get("FFN2_MODE", "f8")  # 'f8' | 'bf16'
KPHASE = int(os.environ.get("KPHASE", "5"))    # build-bisect: 1..5


def build(reps: int = 1):
    nc = bacc.Bacc(None, target_bir_lowering=False)
    xg_d = nc.declare_dram_parameter("xg", [1024, 1024], BF16, isOutput=False)
    xt_d = nc.declare_dram_parameter("xt", [1024, 1024], BF16, isOutput=False)
    wq_d = nc.declare_dram_parameter("wq", [NVQ * 128, 2, 1024], F8, isOutput=False)
    wk_d = nc.declare_dram_parameter("wk", [NVQ * 128, 2, 1024], F8, isOutput=False)
    wv_d = nc.declare_dram_parameter("wv", [NVQ * 128, 2, 1024], F8, isOutput=False)
    wo_d = nc.declare_dram_parameter("wo", [1024, 1024], BF16, isOutput=False)
    w1_d = nc.declare_dram_parameter("w1", [1024, 4096], BF16, isOutput=False)
    if FFN2_MODE == "f8":
        w2_d = nc.declare_dram_parameter("w2", [NV2 * 128, 2, 1024], F8,
                                         isOutput=False)
    else:
        w2_d = nc.declare_dram_parameter("w2", [4096, 1024], BF16,
                                         isOutput=False)
    bq_d = nc.declare_dram_parameter("bq", [128, 8], F32, isOutput=False)
    bk_d = nc.declare_dram_parameter("bk", [128, 8], F32, isOutput=False)
    b1_d = nc.declare_dram_parameter("b1", [128, 32], F32, isOutput=False)
    bv32_d = nc.declare_dram_parameter("bv32", [1, 1024], F32, isOutput=False)
    bout_d = nc.declare_dram_parameter("bout", [1, 1024], F32, isOutput=False)
    b2_d = nc.declare_dram_parameter("b2", [1, 1024], F32, isOutput=False)
    kmask_d = nc.declare_dram_parameter("kmask", [128, 4, 256], BF16,
                                        isOutput=False)
    out_d = nc.declare_dram_parameter("out", [512, 1024], F32, isOutput=True)

    with TileContext(nc) as tc:
        with tc.tile_pool(name="const", bufs=1) as pconst, \
             tc.tile_pool(name="glob", bufs=1) as glob, \
             tc.tile_pool(name="rot", bufs=2) as rot, \
             tc.tile_pool(name="small", bufs=4) as small:
            # ---- constants ----
            identb = pconst.tile([128, 128], TDT, tag="identb", name="identb")
            make_identity(nc, identb[:])
            identb16 = pconst.tile([128, 128], BF16, tag="identb16",
                                   name="identb16")
            make_identity(nc, identb16[:])
            kmask = pconst.tile([128, 4, 256], BF16, tag="kmask", name="kmask")
            nc.sync.dma_start(out=kmask[:], in_=kmask_d[:])
            bq_t = pconst.tile([128, 8], F32, tag="bq", name="bq")
            bk_t = pconst.tile([128, 8], F32, tag="bk", name="bk")
            b1_t = pconst.tile([128, 32], F32, tag="b1", name="b1")
            nc.sync.dma_start(out=bq_t[:], in_=bq_d[:])
            nc.sync.dma_start(out=bk_t[:], in_=bk_d[:])
            nc.sync.dma_start(out=b1_t[:], in_=b1_d[:])
            bv32_bc = pconst.tile([128, 1024], F32, tag="bv32bc", name="bv32bc")
            bout_bc = pconst.tile([128, 1024], F32, tag="boutbc", name="boutbc")
            b2_bc = pconst.tile([128, 1024], F32, tag="b2bc", name="b2bc")
            brow = pconst.tile([1, 1024], F32, tag="brow", name="brow")
            for src, dst in ((bv32_d, bv32_bc), (bout_d, bout_bc), (b2_d, b2_bc)):
                nc.sync.dma_start(out=brow[:], in_=src[:])
                nc.gpsimd.partition_broadcast(dst[:], brow[:])
            eps_t = pconst.tile([128, 1], F32, tag="eps", name="eps")
            nc.vector.memset(eps_t[:], 1e-5)

            xnew = [glob.tile([128, 1024], F32, tag=f"xn{t}", name=f"xn{t}")
                    for t in range(4)]
            if KPHASE < 4:
                for t in range(4):
                    nc.vector.memset(xnew[t][:], 0.0)


            for _rep in range(reps):
                with tc.tile_pool(name="attn", bufs=1) as attn:
                    # hTp[par][j]: [128, 2, 512] fp8 DR pairs of LN1 output
                    hTp = [[attn.tile([128, 2, 512], F8, tag=f"hT{par}_{j}",
                                      name=f"hT{par}_{j}") for j in range(4)]
                           for par in range(2)]
                    # qT[m]: [128=2heads x 64hd, 2par x 256q] bf16
                    qT8 = [attn.tile([128, 512], BF16, tag=f"qT{m}",
                                     name=f"qT{m}") for m in range(8)]
                    # kT[m]: [128, 2par x 512keys] bf16
                    kT8 = [attn.tile([128, 1024], BF16, tag=f"kT{m}",
                                     name=f"kT{m}") for m in range(8)]
                    # Vb[par][kt]: [128 keys, 16 heads, 66] bf16 (col 64:66=VS)
                    Vb = [[attn.tile([128, 16, 66], BF16, tag=f"V{par}_{kt}",
                                     name=f"V{par}_{kt}") for kt in range(4)]
                          for par in range(2)]
                    for par in range(2):
                        for kt in range(4):
                            nc.vector.memset(Vb[par][kt][:, :, 64:66], VS)
                    # oT[d]: [128 dout, 2par x 256 tok] bf16
                    oT8 = [attn.tile([128, 512], BF16, tag=f"oT{d}",
                                     name=f"oT{d}") for d in range(8)]

                    # ---- LN1: stats from xg, normalize host-packed xT ----
                    # stt cols 0..7 = mu*rstd per token tile, 8..15 = rstd
                    stt = attn.tile([128, 16], F32, tag="stt", name="stt")
                    xgr = xg_d.rearrange("(t p) d -> t p d", p=128)
                    xtr = xt_d.rearrange("(s p) d -> s p d", p=128)
                    xg_own = {}
                    with tc.tile_pool(name="halo", bufs=1) as halo, \
                         tc.tile_pool(name="ppst", bufs=1, space="PSUM") as ppst:
                        # prefetch xT slices on the ACT DMA queue (keeps the
                        # sync queue free for weight streams)
                        xts8 = [halo.tile([128, 1024], BF16, tag=f"xts{s}",
                                          name=f"xts{s}") for s in range(8)]
                        for s in range(8):
                            nc.scalar.dma_start(out=xts8[s][:], in_=xtr[s])
                        for t in range(8):
                            par, tt = t // 4, t % 4
                            if tt >= 2:
                                xt = attn.tile([128, 1024], BF16,
                                               tag=f"xg{t}", name=f"xg{t}")
                                xg_own[t] = xt
                            else:
                                xt = halo.tile([128, 1024], BF16,
                                               tag=f"xh{t}", name=f"xh{t}")
                            nc.sync.dma_start(out=xt[:], in_=xgr[t])
                            stats = small.tile(
                                [128, 2, nc.vector.BN_STATS_DIM], F32,
                                tag="stats", name="stats")
                            mv = small.tile([128, nc.vector.BN_AGGR_DIM],
                                            F32, tag="mv", name="mv")
                            for sg in range(2):
                                nc.vector.bn_stats(
                                    out=stats[:, sg, :],
                                    in_=xt[:, sg * 512:(sg + 1) * 512])
                            nc.vector.bn_aggr(out=mv[:], in_=stats[:])
                            nc.scalar.activation(out=stt[:, 8 + t:9 + t],
                                                 in_=mv[:, 1:2],
                                                 func=AF.Sqrt, bias=eps_t[:],
                                                 scale=1.0)
                            nc.vector.reciprocal(out=stt[:, 8 + t:9 + t],
                                                 in_=stt[:, 8 + t:9 + t])
                            nc.vector.tensor_tensor(out=stt[:, t:t + 1],
                                                    in0=mv[:, 0:1],
                                                    in1=stt[:, 8 + t:9 + t],
                                                    op=OP.mult)
                        pst = ppst.tile([16, 128], F32, tag="pst", name="pst")
                        nc.tensor.matmul(pst[:], stt[:], identb[:],
                                         is_transpose=True, start=True,
                                         stop=True)
                        sttT = attn.tile([16, 128], F32, tag="sttT",
                                         name="sttT")
                        nc.scalar.activation(out=sttT[:], in_=pst[:],
                                             func=AF.Copy)
                        statrow = attn.tile([1, 16, 128], F32, tag="statrow",
                                            name="statrow")
                        nc.scalar.dma_start(out=statrow[:], in_=sttT[:])
                        mur_bc = attn.tile([128, 1024], F32, tag="murbc",
                                           name="murbc")
                        r_bc = attn.tile([128, 1024], F32, tag="rbc1",
                                         name="rbc1")
                        nc.gpsimd.partition_broadcast(mur_bc[:],
                                                      statrow[0:1, 0:8, :])
                        nc.gpsimd.partition_broadcast(r_bc[:],
                                                      statrow[0:1, 8:16, :])
                        for s in range(8):
                            t1 = rot.tile([128, 1024], F32, tag="t1",
                                          name="t1")
                            nc.vector.tensor_tensor(out=t1[:],
                                                    in0=xts8[s][:],
                                                    in1=r_bc[:], op=OP.mult)
                            for par in range(2):
                                nc.vector.tensor_tensor(
                                    out=hTp[par][s // 2][:, s % 2, :],
                                    in0=t1[:, par * 512:(par + 1) * 512],
                                    in1=mur_bc[:, par * 512:(par + 1) * 512],
                                    op=OP.subtract)

                    # ---- QKV (fp8 single-plane DR; bf16 outputs) ----
                    if KPHASE >= 2:
                      with tc.tile_pool(name="pqkv", bufs=2, space="PSUM") as pq, \
                           tc.tile_pool(name="wm", bufs=1) as wm:
                        wqt = wm.tile([128, NVQ, 2, 1024], F8, tag="wq",
                                      name="wq")
                        wkt = wm.tile([128, NVQ, 2, 1024], F8, tag="wk",
                                      name="wk")
                        wvt = wm.tile([128, NVQ, 2, 1024], F8, tag="wv",
                                      name="wv")
                        for w_d, w_t in ((wq_d, wqt), (wk_d, wkt), (wv_d, wvt)):
                            nc.sync.dma_start(
                                out=w_t[:],
                                in_=w_d.rearrange("(v p) i c -> p v i c",
                                                  p=128))
                        for m in range(8):
                            for par in range(2):
                                psq = pq.tile([128, 256], F32, tag="psq",
                                              name="psq")
                                for v in range(NVQ):
                                    nc.tensor.matmul(
                                        psq[:],
                                        wqt[:, v, :, m * 128:(m + 1) * 128],
                                        hTp[par][v][:, :, 256:512],
                                        start=(v == 0), stop=(v == NVQ - 1),
                                        perf_mode=PM.DoubleRow)
                                nc.vector.tensor_scalar(
                                    out=qT8[m][:, par * 256:(par + 1) * 256],
                                    in0=psq[:], scalar1=1.0 / WS,
                                    scalar2=bq_t[:, m:m + 1],
                                    op0=OP.mult, op1=OP.add)
                        for m in range(8):
                            for par in range(2):
                                psk = pq.tile([128, 512], F32, tag="psk",
                                              name="psk")
                                for v in range(NVQ):
                                    nc.tensor.matmul(
                                        psk[:],
                                        wkt[:, v, :, m * 128:(m + 1) * 128],
                                        hTp[par][v][:],
                                        start=(v == 0), stop=(v == NVQ - 1),
                                        perf_mode=PM.DoubleRow)
                                nc.vector.tensor_scalar(
                                    out=kT8[m][:, par * 512:(par + 1) * 512],
                                    in0=psk[:], scalar1=1.0 / WS,
                                    scalar2=bk_t[:, m:m + 1],
                                    op0=OP.mult, op1=OP.add)
                        for npass in range(2):
                            for par in range(2):
                                for tt in range(4):
                                    psv = pq.tile([128, 512], F32, tag="psv",
                                                  name="psv")
                                    for v in range(NVQ):
                                        nc.tensor.matmul(
                                            psv[:],
                                            hTp[par][v][:, :,
                                                        tt * 128:(tt + 1) * 128],
                                            wvt[:, v, :,
                                                npass * 512:(npass + 1) * 512],
                                            start=(v == 0), stop=(v == NVQ - 1),
                                            perf_mode=PM.DoubleRow)
                                    nc.vector.tensor_tensor(
                                        out=Vb[par][tt][:, npass * 8:
                                                        (npass + 1) * 8, 0:64],
                                        in0=psv[:].rearrange("p (h e) -> p h e",
                                                             h=8),
                                        in1=bv32_bc[:, npass * 512:(npass + 1) * 512]
                                            .rearrange("p (h e) -> p h e", h=8),
                                        op=OP.add)

                    # ---- attention (bf16 internals, mask on DVE) ----
                    for par in range(2 if KPHASE >= 3 else 0):
                        with tc.tile_pool(name=f"ppS{par}", bufs=2,
                                          space="PSUM") as ppS, \
                             tc.tile_pool(name=f"ppO{par}", bufs=4,
                                          space="PSUM") as ppO, \
                             tc.tile_pool(name=f"ex{par}", bufs=3) as ex:
                            for h in range(16):
                                m, a = h // 2, h % 2
                                pso = ppO.tile([66, 256], F32, tag="pso",
                                               name="pso")
                                pss = ppS.tile([128, 4, 256], F32, tag="pss",
                                               name="pss")
                                for kt in range(4):
                                    nc.tensor.matmul(
                                        pss[:, kt, :],
                                        kT8[m][a * 64:(a + 1) * 64,
                                               par * 512 + kt * 128:
                                               par * 512 + (kt + 1) * 128],
                                        qT8[m][a * 64:(a + 1) * 64,
                                               par * 256:(par + 1) * 256],
                                        start=True, stop=True)
                                expm = ex.tile([128, 4, 256], BF16, tag="expm",
                                               name="expm")
                                nc.scalar.activation(out=expm[:], in_=pss[:],
                                                     func=AF.Exp, scale=SCALE)
                                nc.vector.tensor_tensor(
                                    out=expm[:], in0=expm[:], in1=kmask[:],
                                    op=OP.mult)
                                for kt in range(4):
                                    nc.tensor.matmul(
                                        pso[:],
                                        Vb[par][kt][:, h, :],
                                        expm[:, kt, :],
                                        start=(kt == 0), stop=(kt == 3))
                                rec = small.tile([1, 256], F32, tag="rec",
                                                 name="rec")
                                nc.vector.reciprocal(out=rec[:],
                                                     in_=pso[64:65, :])
                                rbc = small.tile([64, 256], F32,
                                                 tag="rbc", name="rbc")
                                nc.gpsimd.partition_broadcast(rbc[:], rec[:])
                                nc.vector.tensor_tensor(
                                    out=oT8[h // 2][64 * (h % 2):
                                                    64 * (h % 2) + 64,
                                                    par * 256:(par + 1) * 256],
                                    in0=pso[0:64, :],
                                    in1=rbc[:], op=OP.mult)

                    # ---- out-proj (bf16) + residual ----
                    if KPHASE >= 4:
                      with tc.tile_pool(name="pp8", bufs=1, space="PSUM") as pp8, \
                           tc.tile_pool(name="wr2", bufs=3) as wr2:
                        pso_ = [pp8.tile([128, 512], F32, tag=f"po{i}",
                                         name=f"po{i}") for i in range(8)]
                        for v in range(8):
                            wo = wr2.tile([128, 1024], BF16, tag="wo", name="wo")
                            nc.sync.dma_start(
                                out=wo[:], in_=wo_d[v * 128:(v + 1) * 128])
                            for tb in range(4):
                                for npass in range(2):
                                    nc.tensor.matmul(
                                        pso_[tb * 2 + npass][:],
                                        oT8[v][:, tb * 128:(tb + 1) * 128],
                                        wo[:, npass * 512:(npass + 1) * 512],
                                        start=(v == 0), stop=(v == 7))
                        for tb in range(4):
                            xob = small.tile([128, 1024], F32, tag="xob",
                                             bufs=2, name="xob")
                            xg_o = xg_own[(tb // 2) * 4 + 2 + tb % 2]
                            nc.gpsimd.tensor_tensor(out=xob[:], in0=xg_o[:],
                                                    in1=bout_bc[:], op=OP.add)
                            for npass in range(2):
                                nc.vector.tensor_tensor(
                                    out=xnew[tb][:, npass * 512:(npass + 1) * 512],
                                    in0=pso_[tb * 2 + npass][:],
                                    in1=xob[:, npass * 512:(npass + 1) * 512],
                                    op=OP.add)

                # ---- FFN ----
                if KPHASE >= 5:
                  with tc.tile_pool(name="ffn", bufs=1) as ffn:
                    # h2T[s]: [128, 512] bf16 (LN2 output, transposed)
                    h2T = [ffn.tile([128, 512], BF16, tag=f"h2T{s}",
                                    name=f"h2T{s}") for s in range(8)]
                    if FFN2_MODE == "f8":
                        fTp = [ffn.tile([128, 2, 512], F8, tag=f"fT{j}",
                                        name=f"fT{j}") for j in range(16)]
                    else:
                        fTb = [ffn.tile([128, 512], BF16, tag=f"fTb{m}",
                                        name=f"fTb{m}") for m in range(32)]
                    with tc.tile_pool(name="ppT2", bufs=8, space="PSUM") as ppT2:
                        pt4s = [ppT2.tile([128, 4, 128], BF16, tag="pt",
                                          name=f"pt2_{d}") for d in range(8)]
                        for t in range(4):
                            h2 = rot.tile([128, 1024], BF16, tag="h", name="h2")
                            stats = small.tile([128, 2, nc.vector.BN_STATS_DIM],
                                               F32, tag="stats", name="stats")
                            mv = small.tile([128, nc.vector.BN_AGGR_DIM], F32,
                                            tag="mv", name="mv")
                            rstd = small.tile([128, 1], F32, tag="rstd",
                                              name="rstd")
                            for sg in range(2):
                                nc.vector.bn_stats(
                                    out=stats[:, sg, :],
                                    in_=xnew[t][:, sg * 512:(sg + 1) * 512])
                            nc.vector.bn_aggr(out=mv[:], in_=stats[:])
                            nc.scalar.activation(out=rstd[:], in_=mv[:, 1:2],
                                                 func=AF.Sqrt, bias=eps_t[:],
                                                 scale=1.0)
                            nc.vector.reciprocal(out=rstd[:], in_=rstd[:])
                            nc.vector.tensor_scalar(
                                out=h2[:], in0=xnew[t][:], scalar1=mv[:, 0:1],
                                scalar2=rstd[:], op0=OP.subtract, op1=OP.mult)
                            # xnew += b2 (residual base for FFN2, post-stats)
                            nc.gpsimd.tensor_tensor(out=xnew[t][:],
                                                    in0=xnew[t][:],
                                                    in1=b2_bc[:], op=OP.add)
                            for d in range(8):
                                nc.tensor.matmul(
                                    pt4s[d][:, t, :],
                                    h2[:, d * 128:(d + 1) * 128],
                                    identb16[:], is_transpose=True,
                                    start=(t == 0), stop=(t == 3))
                        for d in range(8):
                            nc.scalar.activation(
                                out=h2T[d][:],
                                in_=pt4s[d][:].rearrange("p a b -> p (a b)"),
                                func=AF.Copy)

                    with tc.tile_pool(name="pf1", bufs=4, space="PSUM") as pf1, \
                         tc.tile_pool(name="wm2", bufs=2) as wm2:
                        w1r = w1_d.rearrange("(s p) m -> p s m", p=128)
                        for mc in range(4):
                            w1t = wm2.tile([128, 8, 1024], BF16, tag="w1",
                                           name="w1")
                            nc.sync.dma_start(
                                out=w1t[:],
                                in_=w1r[:, :, mc * 1024:(mc + 1) * 1024])
                            for mi in range(8):
                                m = mc * 8 + mi
                                ps = pf1.tile([128, 512], F32, tag="ps",
                                              name="ps")
                                for s in range(8):
                                    nc.tensor.matmul(
                                        ps[:],
                                        w1t[:, s, mi * 128:(mi + 1) * 128],
                                        h2T[s][:],
                                        start=(s == 0), stop=(s == 7))
                                if FFN2_MODE == "f8":
                                    nc.scalar.activation(
                                        out=fTp[m // 2][:, m % 2, :], in_=ps[:],
                                        func=AF.Gelu, bias=b1_t[:, m:m + 1],
                                        scale=1.0)
                                else:
                                    nc.scalar.activation(
                                        out=fTb[m][:], in_=ps[:],
                                        func=AF.Gelu, bias=b1_t[:, m:m + 1],
                                        scale=1.0)

                    with tc.tile_pool(name="pp8b", bufs=1, space="PSUM") as pp8b, \
                         tc.tile_pool(name="wr3", bufs=3) as wr3:
                        psf = [pp8b.tile([128, 512], F32, tag=f"pf{i}",
                                         name=f"pf{i}") for i in range(8)]
                        if FFN2_MODE == "f8":
                            for j in range(NV2):
                                w2t = wr3.tile([128, 2, 1024], F8, tag="w2",
                                               name="w2")
                                nc.sync.dma_start(
                                    out=w2t[:], in_=w2_d[j * 128:(j + 1) * 128])
                                for tb in range(4):
                                    for npass in range(2):
                                        nc.tensor.matmul(
                                            psf[tb * 2 + npass][:],
                                            fTp[j][:, :, tb * 128:(tb + 1) * 128],
                                            w2t[:, :, npass * 512:(npass + 1) * 512],
                                            start=(j == 0), stop=(j == NV2 - 1),
                                            perf_mode=PM.DoubleRow)
                        else:
                            for j in range(32):
                                w2t = wr3.tile([128, 1024], BF16, tag="w2",
                                               name="w2")
                                nc.sync.dma_start(
                                    out=w2t[:], in_=w2_d[j * 128:(j + 1) * 128])
                                for tb in range(4):
                                    for npass in range(2):
                                        nc.tensor.matmul(
                                            psf[tb * 2 + npass][:],
                                            fTb[j][:, tb * 128:(tb + 1) * 128],
                                            w2t[:, npass * 512:(npass + 1) * 512],
                                            start=(j == 0), stop=(j == 31))
                        f2scale = 1.0 / WS if FFN2_MODE == "f8" else 1.0
                        for tb in range(4):
                            for npass in range(2):
                                if FFN2_MODE == "f8":
                                    tmp = small.tile([128, 512], F32, tag="tmp",
                                                     bufs=2, name="tmpf")
                                    nc.scalar.activation(
                                        out=tmp[:], in_=psf[tb * 2 + npass][:],
                                        func=AF.Copy, scale=f2scale)
                                    src = tmp[:]
                                else:
                                    src = psf[tb * 2 + npass][:]
                                nc.vector.tensor_tensor(
                                    out=xnew[tb][:, npass * 512:(npass + 1) * 512],
                                    in0=src,
                                    in1=xnew[tb][:, npass * 512:(npass + 1) * 512],
                                    op=OP.add)

            for t in range(4):
                nc.sync.dma_start(out=out_d.rearrange("(t p) d -> t p d", p=128)[t],
                                  in_=xnew[t][:])

    nc.compile()
    return nc


# ---------------- host-side packing ----------------

def _q8(a):
    return np.clip(a, -224.0, 224.0).astype(E4)


def _pack_hi(w, nv):
    """w [K, M] fp32 (pre-scaled) -> [nv*128, 2, M] fp8 DR pair planes
    (single plane, no compensation). Pair j covers rows 2j*128..(2j+2)*128."""
    K, M = w.shape
    assert K == nv * 256
    hi = _q8(w)
    out = np.zeros((nv * 128, 2, M), E4)
    for j in range(nv):
        for i in range(2):
            out[j * 128:(j + 1) * 128, i, :] = hi[(2 * j + i) * 128:
                                                  (2 * j + i + 1) * 128, :]
    return out


def _make_kmask(c):
    """bf16 keep-indicator [128 kk, 4 kt, 256 q] for chunk c (parity space)."""
    kk = np.arange(128)[:, None]
    q = np.arange(256)[None, :]
    km = np.zeros((128, 4, 256), np.float32)
    for t in range(4):
        Qg = c * 256 + q
        Kg = c * 256 - 256 + t * 128 + kk
        keep = (Kg >= 0) & (Qg - Kg >= 0) & (Qg - Kg <= 256)
        km[:, t, :] = keep.astype(np.float32)
    return km.astype(ml_dtypes.bfloat16)


def make_in_maps(inputs):
    x = np.asarray(inputs["x"], np.float32)
    ln1g = np.asarray(inputs["ln1_g"], np.float32)
    ln1b = np.asarray(inputs["ln1_b"], np.float32)
    ln2g = np.asarray(inputs["ln2_g"], np.float32)
    ln2b = np.asarray(inputs["ln2_b"], np.float32)
    Wqkv = np.asarray(inputs["Wqkv"], np.float32)
    bqkv = np.asarray(inputs["bqkv"], np.float32)
    Wout = np.asarray(inputs["Wout"], np.float32)
    bout = np.asarray(inputs["bout"], np.float32)
    W1 = np.asarray(inputs["W1"], np.float32)
    b1 = np.asarray(inputs["b1"], np.float32)
    W2 = np.asarray(inputs["W2"], np.float32)
    b2 = np.asarray(inputs["b2"], np.float32)

    # fold LN1 gain/bias into Wqkv/bqkv, LN2 into W1/b1
    Wqkv_f = Wqkv * ln1g[:, None]
    bqkv_f = bqkv + ln1b @ Wqkv
    W1_f = W1 * ln2g[:, None]
    b1_f = b1 + ln2b @ W1

    wq_planes = _pack_hi(Wqkv_f[:, :1024] * WS, NVQ)
    wk_planes = _pack_hi(Wqkv_f[:, 1024:2048] * WS, NVQ)
    wv_planes = _pack_hi(Wqkv_f[:, 2048:] * VS, NVQ)
    bq = bqkv_f[:1024].reshape(8, 128).T.copy()
    bk = bqkv_f[1024:2048].reshape(8, 128).T.copy()
    bv32 = (bqkv_f[2048:] * VS).reshape(1, 1024)
    if FFN2_MODE == "f8":
        w2_planes = _pack_hi(W2 * WS, NV2)
    else:
        w2_planes = W2.astype(ml_dtypes.bfloat16)

    common = {
        "wq": wq_planes, "wk": wk_planes, "wv": wv_planes,
        "wo": Wout.astype(ml_dtypes.bfloat16),
        "w1": W1_f.astype(ml_dtypes.bfloat16),
        "w2": w2_planes,
        "bq": np.ascontiguousarray(bq), "bk": np.ascontiguousarray(bk),
        "b1": np.ascontiguousarray(b1_f.reshape(32, 128).T),
        "bv32": bv32, "bout": bout.reshape(1, 1024), "b2": b2.reshape(1, 1024),
    }
    in_maps = []
    for core in range(8):
        b, c = core // 4, core % 4
        xg = np.zeros((1024, 1024), np.float32)
        for par in range(2):
            i0, i1 = c * 256 - 256, c * 256 + 256
            ii = np.arange(max(i0, 0), i1)
            xg[par * 512 + (ii - i0), :] = x[b, 2 * ii + par, :]
        m = dict(common)
        m["xg"] = xg.astype(ml_dtypes.bfloat16)
        m["xt"] = np.ascontiguousarray(xg.T).astype(ml_dtypes.bfloat16)
        m["kmask"] = _make_kmask(c)
        in_maps.append(m)
    return in_maps


def assemble(results):
    out = np.zeros((B, L, D), np.float32)
    for core in range(8):
        b, c = core // 4, core % 4
        o = results[core]["out"]
        for par in range(2):
            ii = np.arange(c * 256, (c + 1) * 256)
            out[b, 2 * ii + par, :] = o[par * 256:(par + 1) * 256, :]
    return out


_CACHE = {}


def kernel(**inputs):
    """Full-input entry point: shards across 8 NeuronCores, runs the Bass
    kernel SPMD, gathers the full [B, L, D] float32 output."""
    if "nc" not in _CACHE:
        _CACHE["nc"] = build()
    nc = _CACHE["nc"]
    in_maps = make_in_maps(inputs)
    res = run_bass_kernel_spmd(nc, in_maps, list(range(8)))
    return assemble(res.results)


# revision 11
# speedup vs baseline: 1.3770x; 1.0091x over previous
"""Dilated-attention transformer block kernel for TRN2, 8-core SPMD. v3.

Sharding: (batch b in {0,1}) x (sequence chunk c in {0..3}) -> 8 cores.
Each core computes the full block for its 512 tokens, with a 512-token
halo for K/V. Dilation=2 handled by parity-grouping the sequence.

v3 precision plan (vs v2's all-fp8-hi/lo):
  - QKV GEMM: fp8 e4m3 single-plane weights, DoubleRow (1x cost).
  - Attention internals (q/k/v/probs) in bf16: scores contraction is 64
    so bf16 costs the same MMs as DR-fp8, with none of the fp8 noise.
  - Mask applied as a bf16 0/1 multiply on DVE after exp (no mask MMs).
  - out-proj + FFN1 in bf16 (hi/lo fp8 costs the same as bf16, is worse).
  - FFN2: fp8 single-plane DR (FFN2_MODE='bf16' fallback).
"""
import os
import numpy as np
import ml_dtypes
import concourse.bass as bass
from concourse import bacc
import concourse.mybir as mybir
from concourse.tile import TileContext
from concourse.bass_utils import run_bass_kernel_spmd
from concourse.masks import make_identity

dt = mybir.dt
F32, F32R, F8, BF16 = dt.float32, dt.float32r, dt.float8e4, dt.bfloat16
AF = mybir.ActivationFunctionType
OP = mybir.AluOpType
PM = mybir.MatmulPerfMode
E4 = ml_dtypes.float8_e4m3

B, L, D, H, HD = 2, 2048, 1024, 16, 64
SCALE = 1.0 / float(np.sqrt(HD))
WS = 64.0          # fp8 weight quantization scale (power of 2)
VS = 32.0          # Wv quantization scale; V tiles hold 32*v, sum row = 32

NVQ = 4            # qkv DR pairs (single-plane)
NV2 = 16           # ffn2 DR pairs (single-plane)
TDT = F32          # transpose dtype
FFN2_MODE = os.environ.get("FFN2_MODE", "f8")  # 'f8' | 'bf16'
KPHASE = int(os.environ.get("KPHASE", "5"))    # build-bisect: 1..5


def build(reps: int = 1):
    nc = bacc.Bacc(None, target_bir_lowering=False)
    xg_d = nc.declare_dram_parameter("xg", [1024, 1024], BF16, isOutput=False)
    xt_d = nc.declare_dram_parameter("xt", [1024, 1024], BF16, isOutput=False)
    wq_d = nc.declare_dram_parameter("wq", [NVQ * 128, 2, 1024], F8, isOutput=False)
    wk_d = nc.declare_dram_parameter("wk", [NVQ * 128, 2, 1024], F8, isOutput=False)
    wv_d = nc.declare_dram_parameter("wv", [NVQ * 128, 2, 1024], F8, isOutput=False)
    wo_d = nc.declare_dram_parameter("wo", [1024, 1024], BF16, isOutput=False)
    w1_d = nc.declare_dram_parameter("w1", [1024, 4096], BF16, isOutput=False)
    if FFN2_MODE == "f8":
        w2_d = nc.declare_dram_parameter("w2", [NV2 * 128, 2, 1024], F8,
                                         isOutput=False)
    else:
        w2_d = nc.declare_dram_parameter("w2", [4096, 1024], BF16,
                                         isOutput=False)
    bq_d = nc.declare_dram_parameter("bq", [128, 8], F32, isOutput=False)
    bk_d = nc.declare_dram_parameter("bk", [128, 8], F32, isOutput=False)
    b1_d = nc.declare_dram_parameter("b1", [128, 32], F32, isOutput=False)
    bv32_d = nc.declare_dram_parameter("bv32", [1, 1024], F32, isOutput=False)
    bout_d = nc.declare_dram_parameter("bout", [1, 1024], F32, isOutput=False)
    b2_d = nc.declare_dram_parameter("b2", [1, 1024], F32, isOutput=False)
    kmask_d = nc.declare_dram_parameter("kmask", [128, 4, 256], BF16,
                                        isOutput=False)
    out_d = nc.declare_dram_parameter("out", [512, 1024], F32, isOutput=True)

    with TileContext(nc) as tc:
        with tc.tile_pool(name="const", bufs=1) as pconst, \
             tc.tile_pool(name="glob", bufs=1) as glob, \
             tc.tile_pool(name="rot", bufs=2) as rot, \
             tc.tile_pool(name="small", bufs=4) as small:
            # ---- constants ----
            identb = pconst.tile([128, 128], TDT, tag="identb", name="identb")
            make_identity(nc, identb[:])
            kmask = pconst.tile([128, 4, 256], BF16, tag="kmask", name="kmask")
            nc.sync.dma_start(out=kmask[:], in_=kmask_d[:])
            bq_t = pconst.tile([128, 8], F32, tag="bq", name="bq")
            bk_t = pconst.tile([128, 8], F32, tag="bk", name="bk")
            b1_t = pconst.tile([128, 32], F32, tag="b1", name="b1")
            nc.sync.dma_start(out=bq_t[:], in_=bq_d[:])
            nc.sync.dma_start(out=bk_t[:], in_=bk_d[:])
            nc.sync.dma_start(out=b1_t[:], in_=b1_d[:])
            bv32_bc = pconst.tile([128, 1024], F32, tag="bv32bc", name="bv32bc")
            bout_bc = pconst.tile([128, 1024], F32, tag="boutbc", name="boutbc")
            b2_bc = pconst.tile([128, 1024], F32, tag="b2bc", name="b2bc")
            brow = pconst.tile([1, 1024], F32, tag="brow", name="brow")
            for src, dst in ((bv32_d, bv32_bc), (bout_d, bout_bc), (b2_d, b2_bc)):
                nc.sync.dma_start(out=brow[:], in_=src[:])
                nc.gpsimd.partition_broadcast(dst[:], brow[:])
            eps_t = pconst.tile([128, 1], F32, tag="eps", name="eps")
            nc.vector.memset(eps_t[:], 1e-5)

            xnew = [glob.tile([128, 1024], F32, tag=f"xn{t}", name=f"xn{t}")
                    for t in range(4)]
            if KPHASE < 4:
                for t in range(4):
                    nc.vector.memset(xnew[t][:], 0.0)


            for _rep in range(reps):
                with tc.tile_pool(name="attn", bufs=1) as attn:
                    # hTp[par][j]: [128, 2, 512] fp8 DR pairs of LN1 output
                    hTp = [[attn.tile([128, 2, 512], F8, tag=f"hT{par}_{j}",
                                      name=f"hT{par}_{j}") for j in range(4)]
                           for par in range(2)]
                    # qT[m]: [128=2heads x 64hd, 2par x 256q] bf16
                    qT8 = [attn.tile([128, 512], BF16, tag=f"qT{m}",
                                     name=f"qT{m}") for m in range(8)]
                    # kT[m]: [128, 2par x 512keys] bf16
                    kT8 = [attn.tile([128, 1024], BF16, tag=f"kT{m}",
                                     name=f"kT{m}") for m in range(8)]
                    # Vb[par][kt]: [128 keys, 16 heads, 66] bf16 (col 64:66=VS)
                    Vb = [[attn.tile([128, 16, 66], BF16, tag=f"V{par}_{kt}",
                                     name=f"V{par}_{kt}") for kt in range(4)]
                          for par in range(2)]
                    for par in range(2):
                        for kt in range(4):
                            nc.vector.memset(Vb[par][kt][:, :, 64:66], VS)
                    # oT[d]: [128 dout, 2par x 256 tok] bf16
                    oT8 = [attn.tile([128, 512], BF16, tag=f"oT{d}",
                                     name=f"oT{d}") for d in range(8)]

                    # ---- LN1: stats from xg, normalize host-packed xT ----
                    # stt cols 0..7 = mu*rstd per token tile, 8..15 = rstd
                    stt = attn.tile([128, 16], F32, tag="stt", name="stt")
                    xgr = xg_d.rearrange("(t p) d -> t p d", p=128)
                    xtr = xt_d.rearrange("(s p) d -> s p d", p=128)
                    xg_own = {}
                    with tc.tile_pool(name="halo", bufs=1) as halo, \
                         tc.tile_pool(name="ppst", bufs=1, space="PSUM") as ppst:
                        # prefetch xT slices on the ACT DMA queue (keeps the
                        # sync queue free for weight streams)
                        xts8 = [halo.tile([128, 1024], BF16, tag=f"xts{s}",
                                          name=f"xts{s}") for s in range(8)]
                        for s in range(8):
                            nc.scalar.dma_start(out=xts8[s][:], in_=xtr[s])
                        for t in range(8):
                            par, tt = t // 4, t % 4
                            if tt >= 2:
                                xt = attn.tile([128, 1024], BF16,
                                               tag=f"xg{t}", name=f"xg{t}")
                                xg_own[t] = xt
                            else:
                                xt = halo.tile([128, 1024], BF16,
                                               tag=f"xh{t}", name=f"xh{t}")
                            nc.sync.dma_start(out=xt[:], in_=xgr[t])
                            stats = small.tile(
                                [128, 2, nc.vector.BN_STATS_DIM], F32,
                                tag="stats", name="stats")
                            mv = small.tile([128, nc.vector.BN_AGGR_DIM],
                                            F32, tag="mv", name="mv")
                            for sg in range(2):
                                nc.vector.bn_stats(
                                    out=stats[:, sg, :],
                                    in_=xt[:, sg * 512:(sg + 1) * 512])
                            nc.vector.bn_aggr(out=mv[:], in_=stats[:])
                            nc.scalar.activation(out=stt[:, 8 + t:9 + t],
                                                 in_=mv[:, 1:2],
                                                 func=AF.Sqrt, bias=eps_t[:],
                                                 scale=1.0)
                            nc.vector.reciprocal(out=stt[:, 8 + t:9 + t],
                                                 in_=stt[:, 8 + t:9 + t])
                            nc.vector.tensor_tensor(out=stt[:, t:t + 1],
                                                    in0=mv[:, 0:1],
                                                    in1=stt[:, 8 + t:9 + t],
                                                    op=OP.mult)
                        pst = ppst.tile([16, 128], F32, tag="pst", name="pst")
                        nc.tensor.matmul(pst[:], stt[:], identb[:],
                                         is_transpose=True, start=True,
                                         stop=True)
                        sttT = attn.tile([16, 128], F32, tag="sttT",
                                         name="sttT")
                        nc.scalar.activation(out=sttT[:], in_=pst[:],
                                             func=AF.Copy)
                        statrow = attn.tile([1, 16, 128], F32, tag="statrow",
                                            name="statrow")
                        nc.scalar.dma_start(out=statrow[:], in_=sttT[:])
                        mur_bc = attn.tile([128, 1024], F32, tag="murbc",
                                           name="murbc")
                        r_bc = attn.tile([128, 1024], F32, tag="rbc1",
                                         name="rbc1")
                        nc.gpsimd.partition_broadcast(mur_bc[:],
                                                      statrow[0:1, 0:8, :])
                        nc.gpsimd.partition_broadcast(r_bc[:],
                                                      statrow[0:1, 8:16, :])
                        for s in range(8):
                            t1 = rot.tile([128, 1024], F32, tag="t1",
                                          name="t1")
                            nc.vector.tensor_tensor(out=t1[:],
                                                    in0=xts8[s][:],
                                                    in1=r_bc[:], op=OP.mult)
                            for par in range(2):
                                nc.vector.tensor_tensor(
                                    out=hTp[par][s // 2][:, s % 2, :],
                                    in0=t1[:, par * 512:(par + 1) * 512],
                                    in1=mur_bc[:, par * 512:(par + 1) * 512],
                                    op=OP.subtract)

                    # ---- QKV (fp8 single-plane DR; bf16 outputs) ----
                    if KPHASE >= 2:
                      with tc.tile_pool(name="pqkv", bufs=2, space="PSUM") as pq, \
                           tc.tile_pool(name="wm", bufs=1) as wm:
                        wqt = wm.tile([128, NVQ, 2, 1024], F8, tag="wq",
                                      name="wq")
                        wkt = wm.tile([128, NVQ, 2, 1024], F8, tag="wk",
                                      name="wk")
                        wvt = wm.tile([128, NVQ, 2, 1024], F8, tag="wv",
                                      name="wv")
                        for w_d, w_t in ((wq_d, wqt), (wk_d, wkt), (wv_d, wvt)):
                            nc.sync.dma_start(
                                out=w_t[:],
                                in_=w_d.rearrange("(v p) i c -> p v i c",
                                                  p=128))
                        for m in range(8):
                            for par in range(2):
                                psq = pq.tile([128, 256], F32, tag="psq",
                                              name="psq")
                                for v in range(NVQ):
                                    nc.tensor.matmul(
                                        psq[:],
                                        wqt[:, v, :, m * 128:(m + 1) * 128],
                                        hTp[par][v][:, :, 256:512],
                                        start=(v == 0), stop=(v == NVQ - 1),
                                        perf_mode=PM.DoubleRow)
                                nc.vector.tensor_scalar(
                                    out=qT8[m][:, par * 256:(par + 1) * 256],
                                    in0=psq[:], scalar1=1.0 / WS,
                                    scalar2=bq_t[:, m:m + 1],
                                    op0=OP.mult, op1=OP.add)
                        for m in range(8):
                            for par in range(2):
                                psk = pq.tile([128, 512], F32, tag="psk",
                                              name="psk")
                                for v in range(NVQ):
                                    nc.tensor.matmul(
                                        psk[:],
                                        wkt[:, v, :, m * 128:(m + 1) * 128],
                                        hTp[par][v][:],
                                        start=(v == 0), stop=(v == NVQ - 1),
                                        perf_mode=PM.DoubleRow)
                                nc.vector.tensor_scalar(
                                    out=kT8[m][:, par * 512:(par + 1) * 512],
                                    in0=psk[:], scalar1=1.0 / WS,
                                    scalar2=bk_t[:, m:m + 1],
                                    op0=OP.mult, op1=OP.add)
                        for npass in range(2):
                            for par in range(2):
                                for tt in range(4):
                                    psv = pq.tile([128, 512], F32, tag="psv",
                                                  name="psv")
                                    for v in range(NVQ):
                                        nc.tensor.matmul(
                                            psv[:],
                                            hTp[par][v][:, :,
                                                        tt * 128:(tt + 1) * 128],
                                            wvt[:, v, :,
                                                npass * 512:(npass + 1) * 512],
                                            start=(v == 0), stop=(v == NVQ - 1),
                                            perf_mode=PM.DoubleRow)
                                    nc.vector.tensor_tensor(
                                        out=Vb[par][tt][:, npass * 8:
                                                        (npass + 1) * 8, 0:64],
                                        in0=psv[:].rearrange("p (h e) -> p h e",
                                                             h=8),
                                        in1=bv32_bc[:, npass * 512:(npass + 1) * 512]
                                            .rearrange("p (h e) -> p h e", h=8),
                                        op=OP.add)

                    # ---- attention (bf16 internals, mask on DVE) ----
                    for par in range(2 if KPHASE >= 3 else 0):
                        with tc.tile_pool(name=f"ppS{par}", bufs=2,
                                          space="PSUM") as ppS, \
                             tc.tile_pool(name=f"ppO{par}", bufs=4,
                                          space="PSUM") as ppO, \
                             tc.tile_pool(name=f"ex{par}", bufs=3) as ex:
                            for h in range(16):
                                m, a = h // 2, h % 2
                                pso = ppO.tile([66, 256], F32, tag="pso",
                                               name="pso")
                                pss = ppS.tile([128, 4, 256], F32, tag="pss",
                                               name="pss")
                                for kt in range(4):
                                    nc.tensor.matmul(
                                        pss[:, kt, :],
                                        kT8[m][a * 64:(a + 1) * 64,
                                               par * 512 + kt * 128:
                                               par * 512 + (kt + 1) * 128],
                                        qT8[m][a * 64:(a + 1) * 64,
                                               par * 256:(par + 1) * 256],
                                        start=True, stop=True)
                                expm = ex.tile([128, 4, 256], BF16, tag="expm",
                                               name="expm")
                                nc.scalar.activation(out=expm[:], in_=pss[:],
                                                     func=AF.Exp, scale=SCALE)
                                nc.vector.tensor_tensor(
                                    out=expm[:], in0=expm[:], in1=kmask[:],
                                    op=OP.mult)
                                for kt in range(4):
                                    nc.tensor.matmul(
                                        pso[:],
                                        Vb[par][kt][:, h, :],
                                        expm[:, kt, :],
                                        start=(kt == 0), stop=(kt == 3))
                                rec = small.tile([1, 256], F32, tag="rec",
                                                 name="rec")
                                nc.vector.reciprocal(out=rec[:],
                                                     in_=pso[64:65, :])
                                rbc = small.tile([64, 256], F32,
                                                 tag="rbc", name="rbc")
                                nc.gpsimd.partition_broadcast(rbc[:], rec[:])
                                nc.vector.tensor_tensor(
                                    out=oT8[h // 2][64 * (h % 2):
                                                    64 * (h % 2) + 64,
                                                    par * 256:(par + 1) * 256],
                                    in0=pso[0:64, :],
                                    in1=rbc[:], op=OP.mult)

                    # ---- out-proj (bf16) + residual ----
                    if KPHASE >= 4:
                      with tc.tile_pool(name="pp8", bufs=1, space="PSUM") as pp8, \
                           tc.tile_pool(name="wr2", bufs=3) as wr2:
                        pso_ = [pp8.tile([128, 512], F32, tag=f"po{i}",
                                         name=f"po{i}") for i in range(8)]
                        for v in range(8):
                            wo = wr2.tile([128, 1024], BF16, tag="wo", name="wo")
                            nc.sync.dma_start(
                                out=wo[:], in_=wo_d[v * 128:(v + 1) * 128])
                            for tb in range(4):
                                for npass in range(2):
                                    nc.tensor.matmul(
                                        pso_[tb * 2 + npass][:],
                                        oT8[v][:, tb * 128:(tb + 1) * 128],
                                        wo[:, npass * 512:(npass + 1) * 512],
                                        start=(v == 0), stop=(v == 7))
                        for tb in range(4):
                            xob = small.tile([128, 1024], F32, tag="xob",
                                             bufs=2, name="xob")
                            xg_o = xg_own[(tb // 2) * 4 + 2 + tb % 2]
                            nc.gpsimd.tensor_tensor(out=xob[:], in0=xg_o[:],
                                                    in1=bout_bc[:], op=OP.add)
                            for npass in range(2):
                                nc.vector.tensor_tensor(
                                    out=xnew[tb][:, npass * 512:(npass + 1) * 512],
                                    in0=pso_[tb * 2 + npass][:],
                                    in1=xob[:, npass * 512:(npass + 1) * 512],
                                    op=OP.add)

                # ---- FFN ----
                if KPHASE >= 5:
                  with tc.tile_pool(name="ffn", bufs=1) as ffn:
                    # h2T[s]: [128, 512] bf16 (LN2 output, transposed)
                    h2T = [ffn.tile([128, 512], BF16, tag=f"h2T{s}",
                                    name=f"h2T{s}") for s in range(8)]
                    if FFN2_MODE == "f8":
                        fTp = [ffn.tile([128, 2, 512], F8, tag=f"fT{j}",
                                        name=f"fT{j}") for j in range(16)]
                    else:
                        fTb = [ffn.tile([128, 512], BF16, tag=f"fTb{m}",
                                        name=f"fTb{m}") for m in range(32)]
                    with tc.tile_pool(name="ppT2", bufs=8, space="PSUM") as ppT2:
                        pt4s = [ppT2.tile([128, 4, 128], TDT, tag="pt",
                                          name=f"pt2_{d}") for d in range(8)]
                        for t in range(4):
                            h2 = rot.tile([128, 1024], TDT, tag="h", name="h2")
                            stats = small.tile([128, 2, nc.vector.BN_STATS_DIM],
                                               F32, tag="stats", name="stats")
                            mv = small.tile([128, nc.vector.BN_AGGR_DIM], F32,
                                            tag="mv", name="mv")
                            rstd = small.tile([128, 1], F32, tag="rstd",
                                              name="rstd")
                            for sg in range(2):
                                nc.vector.bn_stats(
                                    out=stats[:, sg, :],
                                    in_=xnew[t][:, sg * 512:(sg + 1) * 512])
                            nc.vector.bn_aggr(out=mv[:], in_=stats[:])
                            nc.scalar.activation(out=rstd[:], in_=mv[:, 1:2],
                                                 func=AF.Sqrt, bias=eps_t[:],
                                                 scale=1.0)
                            nc.vector.reciprocal(out=rstd[:], in_=rstd[:])
                            nc.vector.tensor_scalar(
                                out=h2[:], in0=xnew[t][:], scalar1=mv[:, 0:1],
                                scalar2=rstd[:], op0=OP.subtract, op1=OP.mult)
                            # xnew += b2 (residual base for FFN2, post-stats)
                            nc.gpsimd.tensor_tensor(out=xnew[t][:],
                                                    in0=xnew[t][:],
                                                    in1=b2_bc[:], op=OP.add)
                            for d in range(8):
                                nc.tensor.matmul(
                                    pt4s[d][:, t, :],
                                    h2[:, d * 128:(d + 1) * 128],
                                    identb[:], is_transpose=True,
                                    start=(t == 0), stop=(t == 3))
                        for d in range(8):
                            nc.scalar.activation(
                                out=h2T[d][:],
                                in_=pt4s[d][:].rearrange("p a b -> p (a b)"),
                                func=AF.Copy)

                    with tc.tile_pool(name="pf1", bufs=4, space="PSUM") as pf1, \
                         tc.tile_pool(name="wm2", bufs=2) as wm2:
                        w1r = w1_d.rearrange("(s p) m -> p s m", p=128)
                        for mc in range(4):
                            w1t = wm2.tile([128, 8, 1024], BF16, tag="w1",
                                           name="w1")
                            nc.sync.dma_start(
                                out=w1t[:],
                                in_=w1r[:, :, mc * 1024:(mc + 1) * 1024])
                            for mi in range(8):
                                m = mc * 8 + mi
                                ps = pf1.tile([128, 512], F32, tag="ps",
                                              name="ps")
                                for s in range(8):
                                    nc.tensor.matmul(
                                        ps[:],
                                        w1t[:, s, mi * 128:(mi + 1) * 128],
                                        h2T[s][:],
                                        start=(s == 0), stop=(s == 7))
                                if FFN2_MODE == "f8":
                                    nc.scalar.activation(
                                        out=fTp[m // 2][:, m % 2, :], in_=ps[:],
                                        func=AF.Gelu, bias=b1_t[:, m:m + 1],
                                        scale=1.0)
                                else:
                                    nc.scalar.activation(
                                        out=fTb[m][:], in_=ps[:],
                                        func=AF.Gelu, bias=b1_t[:, m:m + 1],
                                        scale=1.0)

                    with tc.tile_pool(name="pp8b", bufs=1, space="PSUM") as pp8b, \
                         tc.tile_pool(name="wr3", bufs=3) as wr3:
                        psf = [pp8b.tile([128, 512], F32, tag=f"pf{i}",
                                         name=f"pf{i}") for i in range(8)]
                        if FFN2_MODE == "f8":
                            for j in range(NV2):
                                w2t = wr3.tile([128, 2, 1024], F8, tag="w2",
                                               name="w2")
                                nc.sync.dma_start(
                                    out=w2t[:], in_=w2_d[j * 128:(j + 1) * 128])
                                for tb in range(4):
                                    for npass in range(2):
                                        nc.tensor.matmul(
                                            psf[tb * 2 + npass][:],
                                            fTp[j][:, :, tb * 128:(tb + 1) * 128],
                                            w2t[:, :, npass * 512:(npass + 1) * 512],
                                            start=(j == 0), stop=(j == NV2 - 1),
                                            perf_mode=PM.DoubleRow)
                        else:
                            for j in range(32):
                                w2t = wr3.tile([128, 1024], BF16, tag="w2",
                                               name="w2")
                                nc.sync.dma_start(
                                    out=w2t[:], in_=w2_d[j * 128:(j + 1) * 128])
                                for tb in range(4):
                                    for npass in range(2):
                                        nc.tensor.matmul(
                                            psf[tb * 2 + npass][:],
                                            fTb[j][:, tb * 128:(tb + 1) * 128],
                                            w2t[:, npass * 512:(npass + 1) * 512],
                                            start=(j == 0), stop=(j == 31))
                        f2scale = 1.0 / WS if FFN2_MODE == "f8" else 1.0
                        for tb in range(4):
                            for npass in range(2):
                                if FFN2_MODE == "f8":
                                    tmp = small.tile([128, 512], F32, tag="tmp",
                                                     bufs=2, name="tmpf")
                                    nc.scalar.activation(
                                        out=tmp[:], in_=psf[tb * 2 + npass][:],
                                        func=AF.Copy, scale=f2scale)
                                    src = tmp[:]
                                else:
                                    src = psf[tb * 2 + npass][:]
                                nc.vector.tensor_tensor(
                                    out=xnew[tb][:, npass * 512:(npass + 1) * 512],
                                    in0=src,
                                    in1=xnew[tb][:, npass * 512:(npass + 1) * 512],
                                    op=OP.add)

            for t in range(4):
                nc.sync.dma_start(out=out_d.rearrange("(t p) d -> t p d", p=128)[t],
                                  in_=xnew[t][:])

    nc.compile()
    return nc


# ---------------- host-side packing ----------------

def _q8(a):
    return np.clip(a, -224.0, 224.0).astype(E4)


def _pack_hi(w, nv):
    """w [K, M] fp32 (pre-scaled) -> [nv*128, 2, M] fp8 DR pair planes
    (single plane, no compensation). Pair j covers rows 2j*128..(2j+2)*128."""
    K, M = w.shape
    assert K == nv * 256
    hi = _q8(w)
    out = np.zeros((nv * 128, 2, M), E4)
    for j in range(nv):
        for i in range(2):
            out[j * 128:(j + 1) * 128, i, :] = hi[(2 * j + i) * 128:
                                                  (2 * j + i + 1) * 128, :]
    return out


def _make_kmask(c):
    """bf16 keep-indicator [128 kk, 4 kt, 256 q] for chunk c (parity space)."""
    kk = np.arange(128)[:, None]
    q = np.arange(256)[None, :]
    km = np.zeros((128, 4, 256), np.float32)
    for t in range(4):
        Qg = c * 256 + q
        Kg = c * 256 - 256 + t * 128 + kk
        keep = (Kg >= 0) & (Qg - Kg >= 0) & (Qg - Kg <= 256)
        km[:, t, :] = keep.astype(np.float32)
    return km.astype(ml_dtypes.bfloat16)


def make_in_maps(inputs):
    x = np.asarray(inputs["x"], np.float32)
    ln1g = np.asarray(inputs["ln1_g"], np.float32)
    ln1b = np.asarray(inputs["ln1_b"], np.float32)
    ln2g = np.asarray(inputs["ln2_g"], np.float32)
    ln2b = np.asarray(inputs["ln2_b"], np.float32)
    Wqkv = np.asarray(inputs["Wqkv"], np.float32)
    bqkv = np.asarray(inputs["bqkv"], np.float32)
    Wout = np.asarray(inputs["Wout"], np.float32)
    bout = np.asarray(inputs["bout"], np.float32)
    W1 = np.asarray(inputs["W1"], np.float32)
    b1 = np.asarray(inputs["b1"], np.float32)
    W2 = np.asarray(inputs["W2"], np.float32)
    b2 = np.asarray(inputs["b2"], np.float32)

    # fold LN1 gain/bias into Wqkv/bqkv, LN2 into W1/b1
    Wqkv_f = Wqkv * ln1g[:, None]
    bqkv_f = bqkv + ln1b @ Wqkv
    W1_f = W1 * ln2g[:, None]
    b1_f = b1 + ln2b @ W1

    wq_planes = _pack_hi(Wqkv_f[:, :1024] * WS, NVQ)
    wk_planes = _pack_hi(Wqkv_f[:, 1024:2048] * WS, NVQ)
    wv_planes = _pack_hi(Wqkv_f[:, 2048:] * VS, NVQ)
    bq = bqkv_f[:1024].reshape(8, 128).T.copy()
    bk = bqkv_f[1024:2048].reshape(8, 128).T.copy()
    bv32 = (bqkv_f[2048:] * VS).reshape(1, 1024)
    if FFN2_MODE == "f8":
        w2_planes = _pack_hi(W2 * WS, NV2)
    else:
        w2_planes = W2.astype(ml_dtypes.bfloat16)

    common = {
        "wq": wq_planes, "wk": wk_planes, "wv": wv_planes,
        "wo": Wout.astype(ml_dtypes.bfloat16),
        "w1": W1_f.astype(ml_dtypes.bfloat16),
        "w2": w2_planes,
        "bq": np.ascontiguousarray(bq), "bk": np.ascontiguousarray(bk),
        "b1": np.ascontiguousarray(b1_f.reshape(32, 128).T),
        "bv32": bv32, "bout": bout.reshape(1, 1024), "b2": b2.reshape(1, 1024),
    }
    in_maps = []
    for core in range(8):
        b, c = core // 4, core % 4
        xg = np.zeros((1024, 1024), np.float32)
        for par in range(2):
            i0, i1 = c * 256 - 256, c * 256 + 256
            ii = np.arange(max(i0, 0), i1)
            xg[par * 512 + (ii - i0), :] = x[b, 2 * ii + par, :]
        m = dict(common)
        m["xg"] = xg.astype(ml_dtypes.bfloat16)
        m["xt"] = np.ascontiguousarray(xg.T).astype(ml_dtypes.bfloat16)
        m["kmask"] = _make_kmask(c)
        in_maps.append(m)
    return in_maps


def assemble(results):
    out = np.zeros((B, L, D), np.float32)
    for core in range(8):
        b, c = core // 4, core % 4
        o = results[core]["out"]
        for par in range(2):
            ii = np.arange(c * 256, (c + 1) * 256)
            out[b, 2 * ii + par, :] = o[par * 256:(par + 1) * 256, :]
    return out


_CACHE = {}


def kernel(**inputs):
    """Full-input entry point: shards across 8 NeuronCores, runs the Bass
    kernel SPMD, gathers the full [B, L, D] float32 output."""
    if "nc" not in _CACHE:
        _CACHE["nc"] = build()
    nc = _CACHE["nc"]
    in_maps = make_in_maps(inputs)
    res = run_bass_kernel_spmd(nc, in_maps, list(range(8)))
    return assemble(res.results)
